# revision 6
# baseline (speedup 1.0000x reference)
"""Trainium2 Bass kernel for nn_BaseFormer (MobileViTv2-style block).

Architecture: 3x3 conv (C=256->256) + BN + SiLU, 1x1 conv C->D=128,
2 x [linear-attention block + SiLU FFN] on 2x2-patchified tokens,
final BN + 1x1 proj D->C + BN.  Input x: [16, 256, 64, 64] fp32.

Strategy: pure data-parallel over batch (2 per core, 8 cores).  All BN
layers are folded into adjacent 1x1/3x3 conv weights on the host; linear
biases that feed only linear ops are deferred ("running offset" delta)
and folded into downstream weights, so the device only applies biases
that feed nonlinearities (SiLU / relu / the final output).

Device layout: channels on partitions, spatial (h*64+w) on the free dim.
The 2x2 patch structure of the attention is handled with strided access
patterns (h%2, w%2) -- patches are never materialized.  The 3x3 conv is
9 shifted 1x1 convs accumulated in PSUM, reading from a host-padded
[66,66] input layout.  Matmuls run as float32r (full fp32 storage,
1 cycle/row on the PE for N>=256).
"""

import os
import numpy as np
from contextlib import ExitStack

import concourse.bass as bass
import concourse.tile as tile
import concourse.mybir as mybir
from concourse.bass_utils import run_bass_kernel_spmd

F32 = mybir.dt.float32
F32R = mybir.dt.float32r
AF = mybir.ActivationFunctionType
ALU = mybir.AluOpType

B, C, H, W = 16, 256, 64, 64
D, FFN = 128, 256
NCORES = 8
NB = B // NCORES          # batches per core = 2
HW = H * W                # 4096
HP = WP = 66              # padded spatial
CH = 1024                 # column chunk (2 PSUM banks)
NCH = HW // CH            # 4
NT = 56                   # weight tiles in the packed wall
NBV = 12                  # bias vector columns

# matmul compute dtype: "f32r" (default) or "f32"
MM_MODE = os.environ.get("BASS_MM_MODE", "f32r")


def _legalize_waits(nc):
    """Walrus codegen in this toolchain can encode at most one sem-wait per
    instruction, and none on Matmult (the fused fp32 weight-load path).
    Hoist excess waits onto standalone InstEventSemaphore instructions
    immediately before the owner, on the same engine queue."""
    for f in nc.m.functions:
        for blk in f.blocks:
            out = []
            changed = False
            for inst in blk.instructions:
                si = inst.sync_info
                tn = type(inst).__name__
                waits = list(si.on_wait) if si is not None and si.on_wait else []
                is_mm = tn == "InstMatmult"
                if waits and (is_mm or len(waits) > 1):
                    keep = [] if is_mm else [waits[0]]
                    moved = waits if is_mm else waits[1:]
                    for j, wv in enumerate(moved):
                        out.append(mybir.InstEventSemaphore(
                            name=f"{inst.name}_hw{j}", engine=inst.engine,
                            ins=[], outs=[],
                            sync_info=mybir.SyncInfo(on_wait=[wv], on_update=[]),
                        ))
                    inst.sync_info = mybir.SyncInfo(
                        on_wait=keep, on_update=list(si.on_update))
                    changed = True
                out.append(inst)
            if changed:
                blk.instructions = out


def _bn_fold(p, eps):
    g = np.asarray(p["gamma"], np.float64)
    be = np.asarray(p["beta"], np.float64)
    mu = np.asarray(p["mean"], np.float64)
    va = np.asarray(p["var"], np.float64)
    s = g / np.sqrt(va + eps)
    return s, be - mu * s


def _prep(x, params):
    """Host-side folding: returns (xpad[B,C,66,66], wall[128,NT,128], bvec[128,NBV])."""
    P = params
    s3, b3 = _bn_fold(P["bn3"], 1e-3)
    w3 = np.asarray(P["conv3_w"], np.float64) * s3[:, None, None, None]
    cw = np.asarray(P["conv1_w"], np.float64)

    wall = np.zeros((128, NT, 128), np.float64)
    bvec = np.zeros((128, NBV), np.float64)

    for tap in range(9):
        dy, dx = tap // 3, tap % 3
        for kt in range(2):
            for co in range(2):
                wall[:, tap * 4 + kt * 2 + co, :] = (
                    w3[co * 128:(co + 1) * 128, kt * 128:(kt + 1) * 128, dy, dx].T
                )
    for kt in range(2):
        wall[:, 36 + kt, :] = cw[:, kt * 128:(kt + 1) * 128].T

    bvec[:, 0] = b3[:128]
    bvec[:, 1] = b3[128:]

    delta = np.zeros(D, np.float64)
    for i, blk in enumerate(P["blocks"]):
        base = 38 + 8 * i
        bb = 2 + 4 * i
        sa, ba = _bn_fold(blk["attn_bn"], 1e-5)
        qkv_w = np.asarray(blk["qkv_w"], np.float64)
        qkv_b = np.asarray(blk["qkv_b"], np.float64)
        qW, kW, vW = qkv_w[0], qkv_w[1:1 + D], qkv_w[1 + D:]
        qWp = qW * sa
        kWp = kW * sa[None, :]
        vWp = vW * sa[None, :]
        kc = kWp @ delta + kW @ ba + qkv_b[1:1 + D]
        vc = vWp @ delta + vW @ ba + qkv_b[1 + D:]
        wall[:, base + 0, :] = qWp[:, None]          # replicated across M
        wall[:, base + 1, :] = kWp.T
        wall[:, base + 2, :] = vWp.T
        ow = np.asarray(blk["out_w"], np.float64)
        ob = np.asarray(blk["out_b"], np.float64)
        wall[:, base + 3, :] = ow.T
        delta2 = delta + ob
        sf, bf = _bn_fold(blk["ffn_bn"], 1e-5)
        f1w = np.asarray(blk["ffn1_w"], np.float64)
        f1b = np.asarray(blk["ffn1_b"], np.float64)
        f1Wp = f1w * sf[None, :]
        c1 = f1w @ bf + f1b + f1Wp @ delta2
        f1T = f1Wp.T                                  # [D, FFN]
        wall[:, base + 4, :] = f1T[:, :128]
        wall[:, base + 5, :] = f1T[:, 128:]
        f2w = np.asarray(blk["ffn2_w"], np.float64)
        f2b = np.asarray(blk["ffn2_b"], np.float64)
        f2T = f2w.T                                   # [FFN, D]
        wall[:, base + 6, :] = f2T[:128, :]
        wall[:, base + 7, :] = f2T[128:, :]
        bvec[:, bb + 0] = kc
        bvec[:, bb + 1] = vc
        bvec[:, bb + 2] = c1[:128]
        bvec[:, bb + 3] = c1[128:]
        delta = delta2 + f2b

    sn, bnb = _bn_fold(P["final_bn"], 1e-5)
    pw = np.asarray(P["proj_w"], np.float64)
    sp_, bp = _bn_fold(P["proj_bn"], 1e-3)
    pw1 = pw * sn[None, :]
    wpr = sp_[:, None] * pw1
    fc = sp_ * (pw1 @ delta + pw @ bnb) + bp
    wall[:, 54, :] = wpr[:128, :].T
    wall[:, 55, :] = wpr[128:, :].T
    bvec[:, 10] = fc[:128]
    bvec[:, 11] = fc[128:]

    xpad = np.pad(np.asarray(x, np.float32), ((0, 0), (0, 0), (1, 1), (1, 1)))
    return xpad, np.ascontiguousarray(wall.astype(np.float32)), np.ascontiguousarray(bvec.astype(np.float32))


def _patch_sem_clear(nc):
    """Tile teardown emits one RANGE_CLEAR ISA op over all allocated sems;
    walrus rejects wide ranges ("ISA wrong length").  Chunk to <=5 sems."""
    import types
    from concourse.bass import compact_to_ranges

    def _chunked(self, sems):
        if not sems:
            return
        sem_nums = [getattr(s, "num", s) for s in sems]
        for r in compact_to_ranges(sem_nums):
            assert self._state.free_isdisjoint(r)
            for i in range(r.start, r.stop, 5):
                sub = range(i, min(i + 5, r.stop))
                self.gpsimd.dma_reset(sub)
                self.gpsimd.sem_clear(sub)
        self._state.prepend_free_semaphores(sem_nums)
        for poison_set in self._tile_sem_poison_stack:
            poison_set.update(sem_nums)

    nc.clear_and_free_semaphores = types.MethodType(_chunked, nc)


def _build_program():
    nc = bass.Bass("TRN2", target_bir_lowering=False, debug=False)
    _patch_sem_clear(nc)
    MD = F32R if MM_MODE == "f32r" else F32
    xpad_d = nc.dram_tensor("xpad", [NB, C, HP, WP], MD, kind="ExternalInput").ap()
    wall_d = nc.dram_tensor("wall", [128, NT, 128], MD, kind="ExternalInput").ap()
    bvec_d = nc.dram_tensor("bvec", [128, NBV], F32, kind="ExternalInput").ap()
    out_d = nc.dram_tensor("out", [NB, C, H, W], F32, kind="ExternalOutput").ap()

    def mm(ap):
        return ap

    with tile.TileContext(nc) as tc, ExitStack() as ctx:
        wp = ctx.enter_context(tc.tile_pool(name="wp", bufs=1))
        xp = ctx.enter_context(tc.tile_pool(name="xp", bufs=1))
        hp = ctx.enter_context(tc.tile_pool(name="hp", bufs=3))
        pp = ctx.enter_context(tc.tile_pool(name="pp", bufs=2))
        bigp = ctx.enter_context(tc.tile_pool(name="bigp", bufs=1))
        smp = ctx.enter_context(tc.tile_pool(name="smp", bufs=2))
        up = ctx.enter_context(tc.tile_pool(name="up", bufs=3))
        ob = ctx.enter_context(tc.tile_pool(name="ob", bufs=2))
        ps = ctx.enter_context(tc.tile_pool(name="ps", bufs=4, space="PSUM"))

        w_sb = wp.tile([128, NT * 128], MD, tag="wall")
        nc.sync.dma_start(w_sb[:], wall_d.rearrange("p t m -> p (t m)"))
        bv = wp.tile([128, NBV], F32, tag="bvec")
        nc.sync.dma_start(bv[:], bvec_d[:])

        def wt(t):
            return mm(w_sb[:, t * 128:(t + 1) * 128])

        for b in range(NB):
            # ---- load padded input (both c-tiles) ----
            xt = []
            for kt in range(2):
                x_sb = xp.tile([128, HP * WP], MD, tag=f"x{kt}")
                nc.sync.dma_start(
                    x_sb[:],
                    xpad_d[b, kt * 128:(kt + 1) * 128].rearrange("c h w -> c (h w)"),
                )
                xt.append(x_sb[:].rearrange("p (h w) -> p h w", h=HP, w=WP))

            # ---- conv3x3 + BN + SiLU, then conv1x1 C->D ----
            p_cur = pp.tile([128, HW], MD, tag="p")
            for ch in range(NCH):
                hts = []
                for co in range(2):
                    acc = ps.tile([128, CH], F32, tag="ps")
                    for tap in range(9):
                        dy, dx = tap // 3, tap % 3
                        for kt in range(2):
                            wtile = wt(tap * 4 + kt * 2 + co)
                            first = tap == 0 and kt == 0
                            last = tap == 8 and kt == 1
                            for blk in range(2):
                                h0 = ch * 16 + blk * 8
                                rhs = xt[kt][:, h0 + dy:h0 + dy + 8, dx:dx + 64]
                                nc.tensor.matmul(
                                    acc[:, blk * 512:(blk + 1) * 512],
                                    wtile, mm(rhs), start=first, stop=last,
                                )
                    ht = hp.tile([128, CH], MD, tag="h")
                    nc.scalar.activation(ht[:], acc[:], AF.Silu, bias=bv[:, co:co + 1])
                    hts.append(ht)
                acc = ps.tile([128, CH], F32, tag="ps")
                for kt in range(2):
                    for blk in range(2):
                        nc.tensor.matmul(
                            acc[:, blk * 512:(blk + 1) * 512],
                            wt(36 + kt), mm(hts[kt][:, blk * 512:(blk + 1) * 512]),
                            start=kt == 0, stop=kt == 1,
                        )
                nc.vector.tensor_copy(p_cur[:, ch * CH:(ch + 1) * CH], acc[:])

            # ---- 2 x (linear attention + FFN) ----
            for blki in range(2):
                base = 38 + 8 * blki
                bb = 2 + 4 * blki
                pin = p_cur

                es = bigp.tile([128, HW], F32, tag="es")
                esv = es[:].rearrange("p (h w) -> p h w", h=H, w=W)
                sm = smp.tile([128, 48], F32, tag="small")
                # sm cols: 0-15 denom partials, 16-31 numer partials,
                #          32-35 denom, 36-39 numer, 40-43 ctx
                for q in range(4):
                    qp = ps.tile([128, CH], F32, tag="ps")
                    for blk in range(2):
                        nc.tensor.matmul(
                            qp[:, blk * 512:(blk + 1) * 512],
                            wt(base + 0),
                            mm(pin[:, q * CH + blk * 512:q * CH + (blk + 1) * 512]),
                            start=True, stop=True,
                        )
                    qv = qp[:].rearrange("p (h w) -> p h w", h=16, w=64)
                    for g in range(4):
                        ph, pw_ = g // 2, g % 2
                        nc.scalar.activation(
                            esv[:, q * 16 + ph:(q + 1) * 16:2, pw_::2],
                            qv[:, ph::2, pw_::2],
                            AF.Exp,
                            accum_out=sm[:, q * 4 + g:q * 4 + g + 1],
                        )
                for q in range(4):
                    kp = ps.tile([128, CH], F32, tag="ps")
                    for blk in range(2):
                        nc.tensor.matmul(
                            kp[:, blk * 512:(blk + 1) * 512],
                            wt(base + 1),
                            mm(pin[:, q * CH + blk * 512:q * CH + (blk + 1) * 512]),
                            start=True, stop=True,
                        )
                    kv = kp[:].rearrange("p (h w) -> p h w", h=16, w=64)
                    for g in range(4):
                        ph, pw_ = g // 2, g % 2
                        scr = smp.tile([128, 256], F32, tag="scr")
                        nc.vector.scalar_tensor_tensor(
                            out=scr[:].rearrange("p (a b) -> p a b", a=8, b=32),
                            in0=kv[:, ph::2, pw_::2], scalar=1.0,
                            in1=esv[:, q * 16 + ph:(q + 1) * 16:2, pw_::2],
                            op0=ALU.mult, op1=ALU.mult,
                            accum_out=sm[:, 16 + q * 4 + g:16 + q * 4 + g + 1],
                        )
                # reduce partials over quarters: view (q,g) -> (g,q), reduce X
                nc.vector.tensor_reduce(
                    sm[:, 32:36],
                    sm[:, 0:16].rearrange("p (q g) -> p g q", q=4, g=4),
                    axis=mybir.AxisListType.X, op=ALU.add,
                )
                nc.vector.tensor_reduce(
                    sm[:, 36:40],
                    sm[:, 16:32].rearrange("p (q g) -> p g q", q=4, g=4),
                    axis=mybir.AxisListType.X, op=ALU.add,
                )
                nc.vector.reciprocal(sm[:, 44:48], sm[:, 32:36])
                nc.vector.tensor_tensor(
                    sm[:, 40:44], sm[:, 36:40], sm[:, 44:48], op=ALU.mult,
                )
                nc.vector.tensor_scalar(
                    out=sm[:, 40:44], in0=sm[:, 40:44],
                    scalar1=bv[:, bb + 0:bb + 1], scalar2=None, op0=ALU.add,
                )

                vr = bigp.tile([128, HW], MD, tag="vr")
                vrv = vr[:].rearrange("p (h w) -> p h w", h=H, w=W)
                for q in range(4):
                    vp = ps.tile([128, CH], F32, tag="ps")
                    for blk in range(2):
                        nc.tensor.matmul(
                            vp[:, blk * 512:(blk + 1) * 512],
                            wt(base + 2),
                            mm(pin[:, q * CH + blk * 512:q * CH + (blk + 1) * 512]),
                            start=True, stop=True,
                        )
                    nc.vector.tensor_scalar(
                        out=vr[:, q * CH:(q + 1) * CH], in0=vp[:],
                        scalar1=bv[:, bb + 1:bb + 2], scalar2=0.0,
                        op0=ALU.add, op1=ALU.max,
                    )
                for g in range(4):
                    ph, pw_ = g // 2, g % 2
                    nc.vector.tensor_scalar(
                        out=vrv[:, ph::2, pw_::2], in0=vrv[:, ph::2, pw_::2],
                        scalar1=sm[:, 40 + g:40 + g + 1], scalar2=None, op0=ALU.mult,
                    )
                p_att = pp.tile([128, HW], MD, tag="p")
                for q in range(4):
                    op_ = ps.tile([128, CH], F32, tag="ps")
                    for blk in range(2):
                        nc.tensor.matmul(
                            op_[:, blk * 512:(blk + 1) * 512],
                            wt(base + 3),
                            mm(vr[:, q * CH + blk * 512:q * CH + (blk + 1) * 512]),
                            start=True, stop=True,
                        )
                    nc.vector.tensor_tensor(
                        p_att[:, q * CH:(q + 1) * CH], op_[:],
                        pin[:, q * CH:(q + 1) * CH], op=ALU.add,
                    )

                # ---- FFN ----
                p_ffn = pp.tile([128, HW], MD, tag="p")
                for ch in range(NCH):
                    cs = slice(ch * CH, (ch + 1) * CH)
                    us = []
                    for m in range(2):
                        fp1 = ps.tile([128, CH], F32, tag="ps")
                        for blk in range(2):
                            nc.tensor.matmul(
                                fp1[:, blk * 512:(blk + 1) * 512],
                                wt(base + 4 + m),
                                mm(p_att[:, ch * CH + blk * 512:ch * CH + (blk + 1) * 512]),
                                start=True, stop=True,
                            )
                        ut = up.tile([128, CH], MD, tag="u")
                        nc.scalar.activation(
                            ut[:], fp1[:], AF.Silu, bias=bv[:, bb + 2 + m:bb + 3 + m],
                        )
                        us.append(ut)
                    fp2 = ps.tile([128, CH], F32, tag="ps")
                    for kt in range(2):
                        for blk in range(2):
                            nc.tensor.matmul(
                                fp2[:, blk * 512:(blk + 1) * 512],
                                wt(base + 6 + kt),
                                mm(us[kt][:, blk * 512:(blk + 1) * 512]),
                                start=kt == 0, stop=kt == 1,
                            )
                    nc.vector.tensor_tensor(
                        p_ffn[:, cs], fp2[:], p_att[:, cs], op=ALU.add,
                    )
                p_cur = p_ffn

            # ---- final proj D->C (+ folded BNs) ----
            for co in range(2):
                for ch in range(NCH):
                    prp = ps.tile([128, CH], F32, tag="ps")
                    for blk in range(2):
                        nc.tensor.matmul(
                            prp[:, blk * 512:(blk + 1) * 512],
                            wt(54 + co),
                            mm(p_cur[:, ch * CH + blk * 512:ch * CH + (blk + 1) * 512]),
                            start=True, stop=True,
                        )
                    ot = ob.tile([128, CH], F32, tag="osb")
                    nc.scalar.activation(
                        ot[:], prp[:], AF.Identity, bias=bv[:, 10 + co:11 + co],
                    )
                    nc.sync.dma_start(
                        out_d[b, co * 128:(co + 1) * 128]
                        .rearrange("c h w -> c (h w)")[:, ch * CH:(ch + 1) * CH],
                        ot[:],
                    )
    _legalize_waits(nc)
    return nc


_CACHED_NC = None
LAST_RESULTS = None


def kernel(x, params):
    global _CACHED_NC, LAST_RESULTS
    x = np.asarray(x, np.float32)
    xpad, wall, bvec = _prep(x, params)

    if _CACHED_NC is None:
        _CACHED_NC = _build_program()
    nc = _CACHED_NC

    in_maps = []
    for core in range(NCORES):
        shard = np.ascontiguousarray(xpad[core * NB:(core + 1) * NB])
        in_maps.append({"xpad": shard, "wall": wall, "bvec": bvec})

    trace = bool(int(os.environ.get("BASS_KERNEL_TRACE", "0")))
    res = run_bass_kernel_spmd(nc, in_maps, list(range(NCORES)), trace=trace)
    LAST_RESULTS = res
    out = np.concatenate([res.results[i]["out"] for i in range(NCORES)], axis=0)
    return out.astype(np.float32)


# revision 13
# speedup vs baseline: 1.1613x; 1.1613x over previous
"""Trainium2 Bass kernel for nn_BaseFormer (MobileViTv2-style block).

Architecture: 3x3 conv (C=256->256) + BN + SiLU, 1x1 conv C->D=128,
2 x [linear-attention block + SiLU FFN] on 2x2-patchified tokens,
final BN + 1x1 proj D->C + BN.  Input x: [16, 256, 64, 64] fp32.

Strategy: pure data-parallel over batch (2 per core, 8 cores).  All BN
layers are folded into adjacent 1x1/3x3 conv weights on the host; linear
biases that feed only linear ops are deferred ("running offset" delta)
and folded into downstream weights, so the device only applies biases
that feed nonlinearities (SiLU / relu / the final output).

Device layout: channels on partitions, spatial (h*64+w) on the free dim.
The 2x2 patch structure of the attention is handled with strided access
patterns (h%2, w%2) -- patches are never materialized.  The 3x3 conv is
9 shifted 1x1 convs accumulated in PSUM, reading from a host-padded
[66,66] input layout.  Matmuls run as float32r (full fp32 storage,
1 cycle/row on the PE for N>=256).
"""

import os
import numpy as np
from contextlib import ExitStack

import concourse.bass as bass
import concourse.tile as tile
import concourse.mybir as mybir
from concourse.bass_utils import run_bass_kernel_spmd

F32 = mybir.dt.float32
F32R = mybir.dt.float32r
AF = mybir.ActivationFunctionType
ALU = mybir.AluOpType

B, C, H, W = 16, 256, 64, 64
D, FFN = 128, 256
NCORES = 8
NB = B // NCORES          # batches per core = 2
HW = H * W                # 4096
HP = WP = 66              # padded spatial
CH = 1024                 # column chunk (2 PSUM banks)
NCH = HW // CH            # 4
NT = 56                   # weight tiles in the packed wall
NBV = 12                  # bias vector columns

# matmul compute dtype: "f32r" (default) or "f32"
MM_MODE = os.environ.get("BASS_MM_MODE", "f32r")
# Pipelined cross-batch emission + in-place residuals showed a HW race
# (NaN on batch 0); default to the verified sequential schedule.
PIPELINE = bool(int(os.environ.get("BASS_PIPELINE", "0")))
INPLACE = bool(int(os.environ.get("BASS_INPLACE", "0"))) and PIPELINE


def _legalize_waits(nc):
    """Walrus codegen in this toolchain can encode at most one sem-wait per
    instruction, and none on Matmult (the fused fp32 weight-load path).
    Hoist excess waits onto standalone InstEventSemaphore instructions
    immediately before the owner, on the same engine queue."""
    for f in nc.m.functions:
        for blk in f.blocks:
            out = []
            changed = False
            for inst in blk.instructions:
                si = inst.sync_info
                tn = type(inst).__name__
                waits = list(si.on_wait) if si is not None and si.on_wait else []
                is_mm = tn == "InstMatmult"
                if waits and (is_mm or len(waits) > 1):
                    keep = [] if is_mm else [waits[0]]
                    moved = waits if is_mm else waits[1:]
                    for j, wv in enumerate(moved):
                        out.append(mybir.InstEventSemaphore(
                            name=f"{inst.name}_hw{j}", engine=inst.engine,
                            ins=[], outs=[],
                            sync_info=mybir.SyncInfo(on_wait=[wv], on_update=[]),
                        ))
                    inst.sync_info = mybir.SyncInfo(
                        on_wait=keep, on_update=list(si.on_update))
                    changed = True
                out.append(inst)
            if changed:
                blk.instructions = out


def _bn_fold(p, eps):
    g = np.asarray(p["gamma"], np.float64)
    be = np.asarray(p["beta"], np.float64)
    mu = np.asarray(p["mean"], np.float64)
    va = np.asarray(p["var"], np.float64)
    s = g / np.sqrt(va + eps)
    return s, be - mu * s


def _prep(x, params):
    """Host-side folding: returns (xpad[B,C,66,66], wall[128,NT,128], bvec[128,NBV])."""
    P = params
    s3, b3 = _bn_fold(P["bn3"], 1e-3)
    w3 = np.asarray(P["conv3_w"], np.float64) * s3[:, None, None, None]
    cw = np.asarray(P["conv1_w"], np.float64)

    wall = np.zeros((128, NT, 128), np.float64)
    bvec = np.zeros((128, NBV), np.float64)

    for tap in range(9):
        dy, dx = tap // 3, tap % 3
        for kt in range(2):
            for co in range(2):
                wall[:, tap * 4 + kt * 2 + co, :] = (
                    w3[co * 128:(co + 1) * 128, kt * 128:(kt + 1) * 128, dy, dx].T
                )
    for kt in range(2):
        wall[:, 36 + kt, :] = cw[:, kt * 128:(kt + 1) * 128].T

    bvec[:, 0] = b3[:128]
    bvec[:, 1] = b3[128:]

    delta = np.zeros(D, np.float64)
    for i, blk in enumerate(P["blocks"]):
        base = 38 + 8 * i
        bb = 2 + 4 * i
        sa, ba = _bn_fold(blk["attn_bn"], 1e-5)
        qkv_w = np.asarray(blk["qkv_w"], np.float64)
        qkv_b = np.asarray(blk["qkv_b"], np.float64)
        qW, kW, vW = qkv_w[0], qkv_w[1:1 + D], qkv_w[1 + D:]
        qWp = qW * sa
        kWp = kW * sa[None, :]
        vWp = vW * sa[None, :]
        kc = kWp @ delta + kW @ ba + qkv_b[1:1 + D]
        vc = vWp @ delta + vW @ ba + qkv_b[1 + D:]
        wall[:, base + 0, :] = qWp[:, None]          # replicated across M
        wall[:, base + 1, :] = kWp.T
        wall[:, base + 2, :] = vWp.T
        ow = np.asarray(blk["out_w"], np.float64)
        ob = np.asarray(blk["out_b"], np.float64)
        wall[:, base + 3, :] = ow.T
        delta2 = delta + ob
        sf, bf = _bn_fold(blk["ffn_bn"], 1e-5)
        f1w = np.asarray(blk["ffn1_w"], np.float64)
        f1b = np.asarray(blk["ffn1_b"], np.float64)
        f1Wp = f1w * sf[None, :]
        c1 = f1w @ bf + f1b + f1Wp @ delta2
        f1T = f1Wp.T                                  # [D, FFN]
        wall[:, base + 4, :] = f1T[:, :128]
        wall[:, base + 5, :] = f1T[:, 128:]
        f2w = np.asarray(blk["ffn2_w"], np.float64)
        f2b = np.asarray(blk["ffn2_b"], np.float64)
        f2T = f2w.T                                   # [FFN, D]
        wall[:, base + 6, :] = f2T[:128, :]
        wall[:, base + 7, :] = f2T[128:, :]
        bvec[:, bb + 0] = kc
        bvec[:, bb + 1] = vc
        bvec[:, bb + 2] = c1[:128]
        bvec[:, bb + 3] = c1[128:]
        delta = delta2 + f2b

    sn, bnb = _bn_fold(P["final_bn"], 1e-5)
    pw = np.asarray(P["proj_w"], np.float64)
    sp_, bp = _bn_fold(P["proj_bn"], 1e-3)
    pw1 = pw * sn[None, :]
    wpr = sp_[:, None] * pw1
    fc = sp_ * (pw1 @ delta + pw @ bnb) + bp
    wall[:, 54, :] = wpr[:128, :].T
    wall[:, 55, :] = wpr[128:, :].T
    bvec[:, 10] = fc[:128]
    bvec[:, 11] = fc[128:]

    xpad = np.pad(np.asarray(x, np.float32), ((0, 0), (0, 0), (1, 1), (1, 1)))
    return xpad, np.ascontiguousarray(wall.astype(np.float32)), np.ascontiguousarray(bvec.astype(np.float32))


def _patch_sem_clear(nc):
    """Tile teardown emits one RANGE_CLEAR ISA op over all allocated sems;
    walrus rejects wide ranges ("ISA wrong length").  Chunk to <=5 sems."""
    import types
    from concourse.bass import compact_to_ranges

    def _chunked(self, sems):
        if not sems:
            return
        sem_nums = [getattr(s, "num", s) for s in sems]
        for r in compact_to_ranges(sem_nums):
            assert self._state.free_isdisjoint(r)
            for i in range(r.start, r.stop, 5):
                sub = range(i, min(i + 5, r.stop))
                self.gpsimd.dma_reset(sub)
                self.gpsimd.sem_clear(sub)
        self._state.prepend_free_semaphores(sem_nums)
        for poison_set in self._tile_sem_poison_stack:
            poison_set.update(sem_nums)

    nc.clear_and_free_semaphores = types.MethodType(_chunked, nc)


def _build_program():
    nc = bass.Bass("TRN2", target_bir_lowering=False, debug=False)
    _patch_sem_clear(nc)
    MD = F32R if MM_MODE == "f32r" else F32
    xpad_d = nc.dram_tensor("xpad", [NB, C, HP, WP], MD, kind="ExternalInput").ap()
    wall_d = nc.dram_tensor("wall", [128, NT, 128], MD, kind="ExternalInput").ap()
    bvec_d = nc.dram_tensor("bvec", [128, NBV], F32, kind="ExternalInput").ap()
    out_d = nc.dram_tensor("out", [NB, C, H, W], F32, kind="ExternalOutput").ap()

    with tile.TileContext(nc) as tc, ExitStack() as ctx:
        wp = ctx.enter_context(tc.tile_pool(name="wp", bufs=1))
        xp = ctx.enter_context(tc.tile_pool(name="xp", bufs=1))
        hp = ctx.enter_context(tc.tile_pool(name="hp", bufs=2))
        pp = ctx.enter_context(tc.tile_pool(name="pp", bufs=(2 if INPLACE else 4) if PIPELINE else 2))
        abig = ctx.enter_context(tc.tile_pool(name="abig", bufs=4 if PIPELINE else 2))
        smp = ctx.enter_context(tc.tile_pool(name="smp", bufs=2))
        up = ctx.enter_context(tc.tile_pool(name="up", bufs=3))
        ob = ctx.enter_context(tc.tile_pool(name="ob", bufs=2))
        ps = ctx.enter_context(tc.tile_pool(name="ps", bufs=4, space="PSUM"))

        w_sb = wp.tile([128, NT * 128], MD, tag="wall")
        wall_flat = wall_d.rearrange("p t m -> p (t m)")
        nc.sync.dma_start(w_sb[:], wall_flat[:])
        bv = wp.tile([128, NBV], F32, tag="bvec")
        nc.sync.dma_start(bv[:], bvec_d[:])

        def wt(t):
            return w_sb[:, t * 128:(t + 1) * 128]

        st = [dict() for _ in range(NB)]

        def load_x(b):
            xt = []
            for kt in range(2):
                x_sb = xp.tile([128, HP * WP], MD, tag=f"x{kt}")
                nc.sync.dma_start(
                    x_sb[:],
                    xpad_d[b, kt * 128:(kt + 1) * 128].rearrange("c h w -> c (h w)"),
                )
                xt.append(x_sb[:].rearrange("p (h w) -> p h w", h=HP, w=WP))
            st[b]["xt"] = xt

        def conv_chunk(b, ch):
            s = st[b]
            if ch == 0:
                s["p"] = pp.tile([128, HW], MD, tag="p", name=f"p_{b}")
            xt = s["xt"]
            hts = []
            for co in range(2):
                acc = ps.tile([128, CH], F32, tag="ps")
                for tap in range(9):
                    for kt in range(2):
                        dy, dx = tap // 3, tap % 3
                        wtile = wt(tap * 4 + kt * 2 + co)
                        first = tap == 0 and kt == 0
                        last = tap == 8 and kt == 1
                        for blk in range(2):
                            h0 = ch * 16 + blk * 8
                            rhs = xt[kt][:, h0 + dy:h0 + dy + 8, dx:dx + 64]
                            nc.tensor.matmul(
                                acc[:, blk * 512:(blk + 1) * 512],
                                wtile, rhs, start=first, stop=last,
                            )
                ht = hp.tile([128, CH], MD, tag="h")
                nc.scalar.activation(ht[:], acc[:], AF.Silu, bias=bv[:, co:co + 1])
                hts.append(ht)
            acc = ps.tile([128, CH], F32, tag="ps")
            for kt in range(2):
                for blk in range(2):
                    nc.tensor.matmul(
                        acc[:, blk * 512:(blk + 1) * 512],
                        wt(36 + kt), hts[kt][:, blk * 512:(blk + 1) * 512],
                        start=kt == 0, stop=kt == 1,
                    )
            nc.vector.tensor_copy(s["p"][:, ch * CH:(ch + 1) * CH], acc[:])

        def attn_a(b, blki):
            # q matmuls (broadcast row) + exp with per-patch accumulated denominators
            s = st[b]
            base = 38 + 8 * blki
            s["es"] = abig.tile([128, HW], F32, tag="abig", name=f"es_{b}_{blki}")
            s["sm"] = smp.tile([128, 48], F32, tag="small", name=f"sm_{b}_{blki}")
            esv = s["es"][:].rearrange("p (h w) -> p h w", h=H, w=W)
            s["esv"] = esv
            pin = s["p"]
            for q in range(4):
                qp = ps.tile([128, CH], F32, tag="ps")
                for blk in range(2):
                    nc.tensor.matmul(
                        qp[:, blk * 512:(blk + 1) * 512], wt(base + 0),
                        pin[:, q * CH + blk * 512:q * CH + (blk + 1) * 512],
                        start=True, stop=True,
                    )
                qv = qp[:].rearrange("p (h w) -> p h w", h=16, w=64)
                for g in range(4):
                    ph, pw_ = g // 2, g % 2
                    nc.scalar.activation(
                        esv[:, q * 16 + ph:(q + 1) * 16:2, pw_::2],
                        qv[:, ph::2, pw_::2], AF.Exp,
                        accum_out=s["sm"][:, q * 4 + g:q * 4 + g + 1],
                    )

        def attn_b(b, blki):
            # k matmuls + fused (k*es) with accumulated numerators; ctx vector
            s = st[b]
            base = 38 + 8 * blki
            bb = 2 + 4 * blki
            pin, sm, esv = s["p"], s["sm"], s["esv"]
            for q in range(4):
                kp = ps.tile([128, CH], F32, tag="ps")
                for blk in range(2):
                    nc.tensor.matmul(
                        kp[:, blk * 512:(blk + 1) * 512], wt(base + 1),
                        pin[:, q * CH + blk * 512:q * CH + (blk + 1) * 512],
                        start=True, stop=True,
                    )
                kv = kp[:].rearrange("p (h w) -> p h w", h=16, w=64)
                for g in range(4):
                    ph, pw_ = g // 2, g % 2
                    scr = smp.tile([128, 256], F32, tag="scr")
                    nc.vector.scalar_tensor_tensor(
                        out=scr[:].rearrange("p (a b) -> p a b", a=8, b=32),
                        in0=kv[:, ph::2, pw_::2], scalar=1.0,
                        in1=esv[:, q * 16 + ph:(q + 1) * 16:2, pw_::2],
                        op0=ALU.mult, op1=ALU.mult,
                        accum_out=sm[:, 16 + q * 4 + g:16 + q * 4 + g + 1],
                    )
            nc.vector.tensor_reduce(
                sm[:, 32:36], sm[:, 0:16].rearrange("p (q g) -> p g q", q=4, g=4),
                axis=mybir.AxisListType.X, op=ALU.add,
            )
            nc.vector.tensor_reduce(
                sm[:, 36:40], sm[:, 16:32].rearrange("p (q g) -> p g q", q=4, g=4),
                axis=mybir.AxisListType.X, op=ALU.add,
            )
            nc.vector.reciprocal(sm[:, 44:48], sm[:, 32:36])
            nc.vector.tensor_tensor(
                sm[:, 40:44], sm[:, 36:40], sm[:, 44:48], op=ALU.mult,
            )
            nc.vector.tensor_scalar(
                out=sm[:, 40:44], in0=sm[:, 40:44],
                scalar1=bv[:, bb + 0:bb + 1], scalar2=None, op0=ALU.add,
            )

        def attn_c(b, blki):
            # v matmuls + relu(v+vc), then per-patch ctx scaling (on GpSimd)
            s = st[b]
            base = 38 + 8 * blki
            bb = 2 + 4 * blki
            pin, sm = s["p"], s["sm"]
            vr = abig.tile([128, HW], MD, tag="abig")
            s["vr"] = vr
            vrv = vr[:].rearrange("p (h w) -> p h w", h=H, w=W)
            for q in range(4):
                vp = ps.tile([128, CH], F32, tag="ps")
                for blk in range(2):
                    nc.tensor.matmul(
                        vp[:, blk * 512:(blk + 1) * 512], wt(base + 2),
                        pin[:, q * CH + blk * 512:q * CH + (blk + 1) * 512],
                        start=True, stop=True,
                    )
                nc.vector.tensor_scalar(
                    out=vr[:, q * CH:(q + 1) * CH], in0=vp[:],
                    scalar1=bv[:, bb + 1:bb + 2], scalar2=0.0,
                    op0=ALU.add, op1=ALU.max,
                )
            for g in range(4):
                ph, pw_ = g // 2, g % 2
                nc.vector.tensor_scalar(
                    out=vrv[:, ph::2, pw_::2], in0=vrv[:, ph::2, pw_::2],
                    scalar1=sm[:, 40 + g:40 + g + 1], scalar2=None, op0=ALU.mult,
                )

        def attn_d(b, blki):
            # output projection + residual
            s = st[b]
            base = 38 + 8 * blki
            pin, vr = s["p"], s["vr"]
            pout = pin if INPLACE else pp.tile([128, HW], MD, tag="p", name=f"patt_{b}_{blki}")
            for q in range(4):
                op_ = ps.tile([128, CH], F32, tag="ps")
                for blk in range(2):
                    nc.tensor.matmul(
                        op_[:, blk * 512:(blk + 1) * 512], wt(base + 3),
                        vr[:, q * CH + blk * 512:q * CH + (blk + 1) * 512],
                        start=True, stop=True,
                    )
                nc.vector.tensor_tensor(
                    pout[:, q * CH:(q + 1) * CH], op_[:],
                    pin[:, q * CH:(q + 1) * CH], op=ALU.add,
                )
            s["p"] = pout

        def ffn_chunk(b, blki, ch):
            s = st[b]
            base = 38 + 8 * blki
            bb = 2 + 4 * blki
            pin = s["p"]
            cs = slice(ch * CH, (ch + 1) * CH)
            us = []
            for m in range(2):
                fp1 = ps.tile([128, CH], F32, tag="ps")
                for blk in range(2):
                    nc.tensor.matmul(
                        fp1[:, blk * 512:(blk + 1) * 512], wt(base + 4 + m),
                        pin[:, ch * CH + blk * 512:ch * CH + (blk + 1) * 512],
                        start=True, stop=True,
                    )
                ut = up.tile([128, CH], MD, tag="u")
                nc.scalar.activation(
                    ut[:], fp1[:], AF.Silu, bias=bv[:, bb + 2 + m:bb + 3 + m],
                )
                us.append(ut)
            fp2 = ps.tile([128, CH], F32, tag="ps")
            for kt in range(2):
                for blk in range(2):
                    nc.tensor.matmul(
                        fp2[:, blk * 512:(blk + 1) * 512], wt(base + 6 + kt),
                        us[kt][:, blk * 512:(blk + 1) * 512],
                        start=kt == 0, stop=kt == 1,
                    )
            if ch == 0 and not INPLACE:
                s["pf"] = pp.tile([128, HW], MD, tag="p", name=f"pf_{b}_{blki}")
            pout = pin if INPLACE else s["pf"]
            nc.vector.tensor_tensor(pout[:, cs], fp2[:], pin[:, cs], op=ALU.add)
            if ch == NCH - 1 and not INPLACE:
                s["p"] = s["pf"]

        def proj_pair(b, pair):
            s = st[b]
            pin = s["p"]
            for j in range(2):
                idx = pair * 2 + j
                co, ch = idx // NCH, idx % NCH
                prp = ps.tile([128, CH], F32, tag="ps")
                for blk in range(2):
                    nc.tensor.matmul(
                        prp[:, blk * 512:(blk + 1) * 512], wt(54 + co),
                        pin[:, ch * CH + blk * 512:ch * CH + (blk + 1) * 512],
                        start=True, stop=True,
                    )
                ot = ob.tile([128, CH], F32, tag="osb")
                nc.scalar.activation(
                    ot[:], prp[:], AF.Identity, bias=bv[:, 10 + co:11 + co],
                )
                nc.sync.dma_start(
                    out_d[b, co * 128:(co + 1) * 128]
                    .rearrange("c h w -> c (h w)")[:, ch * CH:(ch + 1) * CH],
                    ot[:],
                )

        def stages(b):
            q = []
            q.append(lambda b=b: (load_x(b), conv_chunk(b, 0)))
            for ch in range(1, NCH):
                q.append(lambda b=b, ch=ch: conv_chunk(b, ch))
            for blki in range(2):
                q.append(lambda b=b, blki=blki: attn_a(b, blki))
                q.append(lambda b=b, blki=blki: attn_b(b, blki))
                q.append(lambda b=b, blki=blki: attn_c(b, blki))
                q.append(lambda b=b, blki=blki: attn_d(b, blki))
                for ch in range(NCH):
                    q.append(lambda b=b, blki=blki, ch=ch: ffn_chunk(b, blki, ch))
            for pair in range(4):
                q.append(lambda b=b, pair=pair: proj_pair(b, pair))
            return q

        qa, qb = stages(0), stages(1)
        if PIPELINE:
            LAG = 4
            for i in range(LAG):
                qa[i]()
            for i in range(LAG, len(qa)):
                qa[i]()
                qb[i - LAG]()
            for i in range(len(qa) - LAG, len(qb)):
                qb[i]()
        else:
            for q in qa:
                q()
            for q in qb:
                q()

    _legalize_waits(nc)
    return nc


_CACHED_NC = None
LAST_RESULTS = None


def kernel(x, params):
    global _CACHED_NC, LAST_RESULTS
    x = np.asarray(x, np.float32)
    xpad, wall, bvec = _prep(x, params)

    if _CACHED_NC is None:
        _CACHED_NC = _build_program()
    nc = _CACHED_NC

    in_maps = []
    for core in range(NCORES):
        shard = np.ascontiguousarray(xpad[core * NB:(core + 1) * NB])
        in_maps.append({"xpad": shard, "wall": wall, "bvec": bvec})

    trace = bool(int(os.environ.get("BASS_KERNEL_TRACE", "0")))
    res = run_bass_kernel_spmd(nc, in_maps, list(range(NCORES)), trace=trace)
    LAST_RESULTS = res
    out = np.concatenate([res.results[i]["out"] for i in range(NCORES)], axis=0)
    return out.astype(np.float32)


# revision 14
# speedup vs baseline: 1.2225x; 1.0528x over previous
"""Trainium2 Bass kernel for nn_BaseFormer (MobileViTv2-style block).

Architecture: 3x3 conv (C=256->256) + BN + SiLU, 1x1 conv C->D=128,
2 x [linear-attention block + SiLU FFN] on 2x2-patchified tokens,
final BN + 1x1 proj D->C + BN.  Input x: [16, 256, 64, 64] fp32.

Strategy: pure data-parallel over batch (2 per core, 8 cores).  All BN
layers are folded into adjacent 1x1/3x3 conv weights on the host; linear
biases that feed only linear ops are deferred ("running offset" delta)
and folded into downstream weights, so the device only applies biases
that feed nonlinearities (SiLU / relu / the final output).

Device layout: channels on partitions, spatial (h*64+w) on the free dim.
The 2x2 patch structure of the attention is handled with strided access
patterns (h%2, w%2) -- patches are never materialized.  The 3x3 conv is
9 shifted 1x1 convs accumulated in PSUM, reading from a host-padded
[66,66] input layout.  Matmuls run as float32r (full fp32 storage,
1 cycle/row on the PE for N>=256).
"""

import os
import numpy as np
from contextlib import ExitStack

import concourse.bass as bass
import concourse.tile as tile
import concourse.mybir as mybir
from concourse.bass_utils import run_bass_kernel_spmd

F32 = mybir.dt.float32
F32R = mybir.dt.float32r
AF = mybir.ActivationFunctionType
ALU = mybir.AluOpType

B, C, H, W = 16, 256, 64, 64
D, FFN = 128, 256
NCORES = 8
NB = B // NCORES          # batches per core = 2
HW = H * W                # 4096
HP = WP = 66              # padded spatial
CH = 1024                 # column chunk (2 PSUM banks)
NCH = HW // CH            # 4
NT = 56                   # weight tiles in the packed wall
NBV = 12                  # bias vector columns

# matmul compute dtype: "f32r" (default) or "f32"
MM_MODE = os.environ.get("BASS_MM_MODE", "f32r")
# Pipelined cross-batch emission + in-place residuals showed a HW race
# (NaN on batch 0); default to the verified sequential schedule.
PIPELINE = bool(int(os.environ.get("BASS_PIPELINE", "0")))
INPLACE = bool(int(os.environ.get("BASS_INPLACE", "0"))) and PIPELINE


def _legalize_waits(nc):
    """Walrus codegen in this toolchain can encode at most one sem-wait per
    instruction, and none on Matmult (the fused fp32 weight-load path).
    Hoist excess waits onto standalone InstEventSemaphore instructions
    immediately before the owner, on the same engine queue."""
    for f in nc.m.functions:
        for blk in f.blocks:
            out = []
            changed = False
            for inst in blk.instructions:
                si = inst.sync_info
                tn = type(inst).__name__
                waits = list(si.on_wait) if si is not None and si.on_wait else []
                is_mm = tn == "InstMatmult"
                if waits and (is_mm or len(waits) > 1):
                    keep = [] if is_mm else [waits[0]]
                    moved = waits if is_mm else waits[1:]
                    for j, wv in enumerate(moved):
                        out.append(mybir.InstEventSemaphore(
                            name=f"{inst.name}_hw{j}", engine=inst.engine,
                            ins=[], outs=[],
                            sync_info=mybir.SyncInfo(on_wait=[wv], on_update=[]),
                        ))
                    inst.sync_info = mybir.SyncInfo(
                        on_wait=keep, on_update=list(si.on_update))
                    changed = True
                out.append(inst)
            if changed:
                blk.instructions = out


def _bn_fold(p, eps):
    g = np.asarray(p["gamma"], np.float64)
    be = np.asarray(p["beta"], np.float64)
    mu = np.asarray(p["mean"], np.float64)
    va = np.asarray(p["var"], np.float64)
    s = g / np.sqrt(va + eps)
    return s, be - mu * s


def _prep(x, params):
    """Host-side folding: returns (xpad[B,C,66,66], wall[128,NT,128], bvec[128,NBV])."""
    P = params
    s3, b3 = _bn_fold(P["bn3"], 1e-3)
    w3 = np.asarray(P["conv3_w"], np.float64) * s3[:, None, None, None]
    cw = np.asarray(P["conv1_w"], np.float64)

    wall = np.zeros((128, NT, 128), np.float64)
    bvec = np.zeros((128, NBV), np.float64)

    for tap in range(9):
        dy, dx = tap // 3, tap % 3
        for kt in range(2):
            for co in range(2):
                wall[:, tap * 4 + kt * 2 + co, :] = (
                    w3[co * 128:(co + 1) * 128, kt * 128:(kt + 1) * 128, dy, dx].T
                )
    for kt in range(2):
        wall[:, 36 + kt, :] = cw[:, kt * 128:(kt + 1) * 128].T

    bvec[:, 0] = b3[:128]
    bvec[:, 1] = b3[128:]

    delta = np.zeros(D, np.float64)
    for i, blk in enumerate(P["blocks"]):
        base = 38 + 8 * i
        bb = 2 + 4 * i
        sa, ba = _bn_fold(blk["attn_bn"], 1e-5)
        qkv_w = np.asarray(blk["qkv_w"], np.float64)
        qkv_b = np.asarray(blk["qkv_b"], np.float64)
        qW, kW, vW = qkv_w[0], qkv_w[1:1 + D], qkv_w[1 + D:]
        qWp = qW * sa
        kWp = kW * sa[None, :]
        vWp = vW * sa[None, :]
        kc = kWp @ delta + kW @ ba + qkv_b[1:1 + D]
        vc = vWp @ delta + vW @ ba + qkv_b[1 + D:]
        wall[:, base + 0, :] = qWp[:, None]          # replicated across M
        wall[:, base + 1, :] = kWp.T
        wall[:, base + 2, :] = vWp.T
        ow = np.asarray(blk["out_w"], np.float64)
        ob = np.asarray(blk["out_b"], np.float64)
        wall[:, base + 3, :] = ow.T
        delta2 = delta + ob
        sf, bf = _bn_fold(blk["ffn_bn"], 1e-5)
        f1w = np.asarray(blk["ffn1_w"], np.float64)
        f1b = np.asarray(blk["ffn1_b"], np.float64)
        f1Wp = f1w * sf[None, :]
        c1 = f1w @ bf + f1b + f1Wp @ delta2
        f1T = f1Wp.T                                  # [D, FFN]
        wall[:, base + 4, :] = f1T[:, :128]
        wall[:, base + 5, :] = f1T[:, 128:]
        f2w = np.asarray(blk["ffn2_w"], np.float64)
        f2b = np.asarray(blk["ffn2_b"], np.float64)
        f2T = f2w.T                                   # [FFN, D]
        wall[:, base + 6, :] = f2T[:128, :]
        wall[:, base + 7, :] = f2T[128:, :]
        bvec[:, bb + 0] = kc
        bvec[:, bb + 1] = vc
        bvec[:, bb + 2] = c1[:128]
        bvec[:, bb + 3] = c1[128:]
        delta = delta2 + f2b

    sn, bnb = _bn_fold(P["final_bn"], 1e-5)
    pw = np.asarray(P["proj_w"], np.float64)
    sp_, bp = _bn_fold(P["proj_bn"], 1e-3)
    pw1 = pw * sn[None, :]
    wpr = sp_[:, None] * pw1
    fc = sp_ * (pw1 @ delta + pw @ bnb) + bp
    wall[:, 54, :] = wpr[:128, :].T
    wall[:, 55, :] = wpr[128:, :].T
    bvec[:, 10] = fc[:128]
    bvec[:, 11] = fc[128:]

    xpad = np.pad(np.asarray(x, np.float32), ((0, 0), (0, 0), (1, 1), (1, 1)))
    return xpad, np.ascontiguousarray(wall.astype(np.float32)), np.ascontiguousarray(bvec.astype(np.float32))


def _patch_sem_clear(nc):
    """Tile teardown emits one RANGE_CLEAR ISA op over all allocated sems;
    walrus rejects wide ranges ("ISA wrong length").  Chunk to <=5 sems."""
    import types
    from concourse.bass import compact_to_ranges

    def _chunked(self, sems):
        if not sems:
            return
        sem_nums = [getattr(s, "num", s) for s in sems]
        for r in compact_to_ranges(sem_nums):
            assert self._state.free_isdisjoint(r)
            for i in range(r.start, r.stop, 5):
                sub = range(i, min(i + 5, r.stop))
                self.gpsimd.dma_reset(sub)
                self.gpsimd.sem_clear(sub)
        self._state.prepend_free_semaphores(sem_nums)
        for poison_set in self._tile_sem_poison_stack:
            poison_set.update(sem_nums)

    nc.clear_and_free_semaphores = types.MethodType(_chunked, nc)


def _build_program():
    nc = bass.Bass("TRN2", target_bir_lowering=False, debug=False)
    _patch_sem_clear(nc)
    MD = F32R if MM_MODE == "f32r" else F32
    xpad_d = nc.dram_tensor("xpad", [NB, C, HP, WP], MD, kind="ExternalInput").ap()
    wall_d = nc.dram_tensor("wall", [128, NT, 128], MD, kind="ExternalInput").ap()
    bvec_d = nc.dram_tensor("bvec", [128, NBV], F32, kind="ExternalInput").ap()
    out_d = nc.dram_tensor("out", [NB, C, H, W], F32, kind="ExternalOutput").ap()

    with tile.TileContext(nc) as tc, ExitStack() as ctx:
        wp = ctx.enter_context(tc.tile_pool(name="wp", bufs=1))
        xp = ctx.enter_context(tc.tile_pool(name="xp", bufs=1))
        hp = ctx.enter_context(tc.tile_pool(name="hp", bufs=3))
        pp = ctx.enter_context(tc.tile_pool(name="pp", bufs=(2 if INPLACE else 4) if PIPELINE else 2))
        abig = ctx.enter_context(tc.tile_pool(name="abig", bufs=4 if PIPELINE else 3))
        smp = ctx.enter_context(tc.tile_pool(name="smp", bufs=4))
        up = ctx.enter_context(tc.tile_pool(name="up", bufs=3))
        ob = ctx.enter_context(tc.tile_pool(name="ob", bufs=3))
        ps = ctx.enter_context(tc.tile_pool(name="ps", bufs=4, space="PSUM"))

        w_sb = wp.tile([128, NT * 128], MD, tag="wall")
        wall_flat = wall_d.rearrange("p t m -> p (t m)")
        nc.sync.dma_start(w_sb[:], wall_flat[:])
        bv = wp.tile([128, NBV], F32, tag="bvec")
        nc.sync.dma_start(bv[:], bvec_d[:])

        def wt(t):
            return w_sb[:, t * 128:(t + 1) * 128]

        st = [dict() for _ in range(NB)]

        def load_x(b):
            xt = []
            for kt in range(2):
                x_sb = xp.tile([128, HP * WP], MD, tag=f"x{kt}")
                nc.sync.dma_start(
                    x_sb[:],
                    xpad_d[b, kt * 128:(kt + 1) * 128].rearrange("c h w -> c (h w)"),
                )
                xt.append(x_sb[:].rearrange("p (h w) -> p h w", h=HP, w=WP))
            st[b]["xt"] = xt

        def conv_chunk(b, ch):
            s = st[b]
            if ch == 0:
                s["p"] = pp.tile([128, HW], MD, tag="p", name=f"p_{b}")
            xt = s["xt"]
            hts = []
            for co in range(2):
                acc = ps.tile([128, CH], F32, tag="ps")
                for tap in range(9):
                    for kt in range(2):
                        dy, dx = tap // 3, tap % 3
                        wtile = wt(tap * 4 + kt * 2 + co)
                        first = tap == 0 and kt == 0
                        last = tap == 8 and kt == 1
                        for blk in range(2):
                            h0 = ch * 16 + blk * 8
                            rhs = xt[kt][:, h0 + dy:h0 + dy + 8, dx:dx + 64]
                            nc.tensor.matmul(
                                acc[:, blk * 512:(blk + 1) * 512],
                                wtile, rhs, start=first, stop=last,
                            )
                ht = hp.tile([128, CH], MD, tag="h")
                nc.scalar.activation(ht[:], acc[:], AF.Silu, bias=bv[:, co:co + 1])
                hts.append(ht)
            acc = ps.tile([128, CH], F32, tag="ps")
            for kt in range(2):
                for blk in range(2):
                    nc.tensor.matmul(
                        acc[:, blk * 512:(blk + 1) * 512],
                        wt(36 + kt), hts[kt][:, blk * 512:(blk + 1) * 512],
                        start=kt == 0, stop=kt == 1,
                    )
            nc.vector.tensor_copy(s["p"][:, ch * CH:(ch + 1) * CH], acc[:])

        def attn_a(b, blki):
            # q matmuls (broadcast row) + exp with per-patch accumulated denominators
            s = st[b]
            base = 38 + 8 * blki
            s["es"] = abig.tile([128, HW], F32, tag="abig", name=f"es_{b}_{blki}")
            s["sm"] = smp.tile([128, 48], F32, tag="small", name=f"sm_{b}_{blki}")
            esv = s["es"][:].rearrange("p (h w) -> p h w", h=H, w=W)
            s["esv"] = esv
            pin = s["p"]
            for q in range(4):
                qp = ps.tile([128, CH], F32, tag="ps")
                for blk in range(2):
                    nc.tensor.matmul(
                        qp[:, blk * 512:(blk + 1) * 512], wt(base + 0),
                        pin[:, q * CH + blk * 512:q * CH + (blk + 1) * 512],
                        start=True, stop=True,
                    )
                qv = qp[:].rearrange("p (h w) -> p h w", h=16, w=64)
                for g in range(4):
                    ph, pw_ = g // 2, g % 2
                    nc.scalar.activation(
                        esv[:, q * 16 + ph:(q + 1) * 16:2, pw_::2],
                        qv[:, ph::2, pw_::2], AF.Exp,
                        accum_out=s["sm"][:, q * 4 + g:q * 4 + g + 1],
                    )

        def attn_b(b, blki):
            # k matmuls + fused (k*es) with accumulated numerators; ctx vector
            s = st[b]
            base = 38 + 8 * blki
            bb = 2 + 4 * blki
            pin, sm, esv = s["p"], s["sm"], s["esv"]
            for q in range(4):
                kp = ps.tile([128, CH], F32, tag="ps")
                for blk in range(2):
                    nc.tensor.matmul(
                        kp[:, blk * 512:(blk + 1) * 512], wt(base + 1),
                        pin[:, q * CH + blk * 512:q * CH + (blk + 1) * 512],
                        start=True, stop=True,
                    )
                kv = kp[:].rearrange("p (h w) -> p h w", h=16, w=64)
                for g in range(4):
                    ph, pw_ = g // 2, g % 2
                    scr = smp.tile([128, 256], F32, tag="scr")
                    nc.vector.scalar_tensor_tensor(
                        out=scr[:].rearrange("p (a b) -> p a b", a=8, b=32),
                        in0=kv[:, ph::2, pw_::2], scalar=1.0,
                        in1=esv[:, q * 16 + ph:(q + 1) * 16:2, pw_::2],
                        op0=ALU.mult, op1=ALU.mult,
                        accum_out=sm[:, 16 + q * 4 + g:16 + q * 4 + g + 1],
                    )
            nc.vector.tensor_reduce(
                sm[:, 32:36], sm[:, 0:16].rearrange("p (q g) -> p g q", q=4, g=4),
                axis=mybir.AxisListType.X, op=ALU.add,
            )
            nc.vector.tensor_reduce(
                sm[:, 36:40], sm[:, 16:32].rearrange("p (q g) -> p g q", q=4, g=4),
                axis=mybir.AxisListType.X, op=ALU.add,
            )
            nc.vector.reciprocal(sm[:, 44:48], sm[:, 32:36])
            nc.vector.tensor_tensor(
                sm[:, 40:44], sm[:, 36:40], sm[:, 44:48], op=ALU.mult,
            )
            nc.vector.tensor_scalar(
                out=sm[:, 40:44], in0=sm[:, 40:44],
                scalar1=bv[:, bb + 0:bb + 1], scalar2=None, op0=ALU.add,
            )

        def attn_c(b, blki):
            # v matmuls + relu(v+vc), then per-patch ctx scaling (on GpSimd)
            s = st[b]
            base = 38 + 8 * blki
            bb = 2 + 4 * blki
            pin, sm = s["p"], s["sm"]
            vr = abig.tile([128, HW], MD, tag="abig")
            s["vr"] = vr
            vrv = vr[:].rearrange("p (h w) -> p h w", h=H, w=W)
            for q in range(4):
                vp = ps.tile([128, CH], F32, tag="ps")
                for blk in range(2):
                    nc.tensor.matmul(
                        vp[:, blk * 512:(blk + 1) * 512], wt(base + 2),
                        pin[:, q * CH + blk * 512:q * CH + (blk + 1) * 512],
                        start=True, stop=True,
                    )
                nc.vector.tensor_scalar(
                    out=vr[:, q * CH:(q + 1) * CH], in0=vp[:],
                    scalar1=bv[:, bb + 1:bb + 2], scalar2=0.0,
                    op0=ALU.add, op1=ALU.max,
                )
            for g in range(4):
                ph, pw_ = g // 2, g % 2
                nc.vector.tensor_scalar(
                    out=vrv[:, ph::2, pw_::2], in0=vrv[:, ph::2, pw_::2],
                    scalar1=sm[:, 40 + g:40 + g + 1], scalar2=None, op0=ALU.mult,
                )

        def attn_d(b, blki):
            # output projection + residual
            s = st[b]
            base = 38 + 8 * blki
            pin, vr = s["p"], s["vr"]
            pout = pin if INPLACE else pp.tile([128, HW], MD, tag="p", name=f"patt_{b}_{blki}")
            for q in range(4):
                op_ = ps.tile([128, CH], F32, tag="ps")
                for blk in range(2):
                    nc.tensor.matmul(
                        op_[:, blk * 512:(blk + 1) * 512], wt(base + 3),
                        vr[:, q * CH + blk * 512:q * CH + (blk + 1) * 512],
                        start=True, stop=True,
                    )
                nc.vector.tensor_tensor(
                    pout[:, q * CH:(q + 1) * CH], op_[:],
                    pin[:, q * CH:(q + 1) * CH], op=ALU.add,
                )
            s["p"] = pout

        def ffn_chunk(b, blki, ch):
            s = st[b]
            base = 38 + 8 * blki
            bb = 2 + 4 * blki
            pin = s["p"]
            cs = slice(ch * CH, (ch + 1) * CH)
            us = []
            for m in range(2):
                fp1 = ps.tile([128, CH], F32, tag="ps")
                for blk in range(2):
                    nc.tensor.matmul(
                        fp1[:, blk * 512:(blk + 1) * 512], wt(base + 4 + m),
                        pin[:, ch * CH + blk * 512:ch * CH + (blk + 1) * 512],
                        start=True, stop=True,
                    )
                ut = up.tile([128, CH], MD, tag="u")
                nc.scalar.activation(
                    ut[:], fp1[:], AF.Silu, bias=bv[:, bb + 2 + m:bb + 3 + m],
                )
                us.append(ut)
            fp2 = ps.tile([128, CH], F32, tag="ps")
            for kt in range(2):
                for blk in range(2):
                    nc.tensor.matmul(
                        fp2[:, blk * 512:(blk + 1) * 512], wt(base + 6 + kt),
                        us[kt][:, blk * 512:(blk + 1) * 512],
                        start=kt == 0, stop=kt == 1,
                    )
            if ch == 0 and not INPLACE:
                s["pf"] = pp.tile([128, HW], MD, tag="p", name=f"pf_{b}_{blki}")
            pout = pin if INPLACE else s["pf"]
            nc.vector.tensor_tensor(pout[:, cs], fp2[:], pin[:, cs], op=ALU.add)
            if ch == NCH - 1 and not INPLACE:
                s["p"] = s["pf"]

        def proj_pair(b, pair):
            s = st[b]
            pin = s["p"]
            for j in range(2):
                idx = pair * 2 + j
                co, ch = idx // NCH, idx % NCH
                prp = ps.tile([128, CH], F32, tag="ps")
                for blk in range(2):
                    nc.tensor.matmul(
                        prp[:, blk * 512:(blk + 1) * 512], wt(54 + co),
                        pin[:, ch * CH + blk * 512:ch * CH + (blk + 1) * 512],
                        start=True, stop=True,
                    )
                ot = ob.tile([128, CH], F32, tag="osb")
                nc.scalar.activation(
                    ot[:], prp[:], AF.Identity, bias=bv[:, 10 + co:11 + co],
                )
                nc.sync.dma_start(
                    out_d[b, co * 128:(co + 1) * 128]
                    .rearrange("c h w -> c (h w)")[:, ch * CH:(ch + 1) * CH],
                    ot[:],
                )

        def stages(b):
            q = []
            q.append(lambda b=b: (load_x(b), conv_chunk(b, 0)))
            for ch in range(1, NCH):
                q.append(lambda b=b, ch=ch: conv_chunk(b, ch))
            for blki in range(2):
                q.append(lambda b=b, blki=blki: attn_a(b, blki))
                q.append(lambda b=b, blki=blki: attn_b(b, blki))
                q.append(lambda b=b, blki=blki: attn_c(b, blki))
                q.append(lambda b=b, blki=blki: attn_d(b, blki))
                for ch in range(NCH):
                    q.append(lambda b=b, blki=blki, ch=ch: ffn_chunk(b, blki, ch))
            for pair in range(4):
                q.append(lambda b=b, pair=pair: proj_pair(b, pair))
            return q

        qa, qb = stages(0), stages(1)
        if PIPELINE:
            LAG = 4
            for i in range(LAG):
                qa[i]()
            for i in range(LAG, len(qa)):
                qa[i]()
                qb[i - LAG]()
            for i in range(len(qa) - LAG, len(qb)):
                qb[i]()
        else:
            for q in qa:
                q()
            for q in qb:
                q()

    _legalize_waits(nc)
    return nc


_CACHED_NC = None
LAST_RESULTS = None


def kernel(x, params):
    global _CACHED_NC, LAST_RESULTS
    x = np.asarray(x, np.float32)
    xpad, wall, bvec = _prep(x, params)

    if _CACHED_NC is None:
        _CACHED_NC = _build_program()
    nc = _CACHED_NC

    in_maps = []
    for core in range(NCORES):
        shard = np.ascontiguousarray(xpad[core * NB:(core + 1) * NB])
        in_maps.append({"xpad": shard, "wall": wall, "bvec": bvec})

    trace = bool(int(os.environ.get("BASS_KERNEL_TRACE", "0")))
    res = run_bass_kernel_spmd(nc, in_maps, list(range(NCORES)), trace=trace)
    LAST_RESULTS = res
    out = np.concatenate([res.results[i]["out"] for i in range(NCORES)], axis=0)
    return out.astype(np.float32)
